# revision 30
# baseline (speedup 1.0000x reference)
"""CostVolumeLayer Trainium2 kernel.

Problem: src, tgt [B=8, C=128, H=160, W=288] fp32.
out[b, k, y, x] = (1/C) * sum_c src[b,c,y,x] * tgt[b,c,y+dy_k,x+dx_k]
for the 81 displacements (dy,dx) in [-4,4]^2 (torch CostVolume channel
order), with zero padding outside the image. ~127-130 us HW exec across
8 NeuronCores (staged baseline: 241 us).

Strategy (data-parallel over batch, one batch per NeuronCore):
  - Tiles of 16x8 = 128 src positions (partition p = ylocal*8 + xlocal).
  - Each tile = 4 col-grouped matmuls (tile_position=(0,32j)): group j
    covers ylocal in [4j, 4j+4) (partitions 32j..32j+32) and computes
    only that group's 12x16 = 192-column band of the tgt window (rows
    16s+4j..+12 of the padded tgt, cols 8t..8t+16) via a 2-D rhs access
    pattern into the SBUF-resident padded tgt. This cuts the written
    Gram from the full 24x16=384 window to 192 cols per position (write
    amplification 2.37x instead of 4.7x); the 4 matmuls of a tile run
    concurrently in distinct 32-column strips of the PE array.
  - Two position-tiles accumulate into one 2 KiB PSUM bank; DVE and ACT
    alternate evacuating [128, 384] pairs (x 1/C, cast bf16) so per-op
    overhead is paid half as often and the engines run concurrently.
  - The banded Gram is DMA'd to DRAM fully contiguously (6.9 KB runs per
    partition per strip). The host de-shears it into [B, 81, H, W]
    (the per-partition diagonal gather is not expressible as a uniform
    access pattern on any engine or DMA descriptor).
  - Inputs are cast to bf16 on the host (halves HBM read traffic); PSUM
    accumulation is fp32. tgt is x-padded on the host and its 4 top/
    bottom pad rows are memset on-device, keeping every chunk DMA
    row-contiguous per partition.

Queue/pipelining choices (from perfetto traces):
  - src strips on the sync HWDGE ring (reads only - a queued write on
    the same FIFO ring would delay the next strip load); tgt chunks on
    the scalar HWDGE ring, 8 rows each, first 6 issued up front and the
    rest trickled 2 per strip so the ACT engine's queue is not blocked
    ahead of its first evacuation; output writes on SWDGE (gpsimd) for
    the first half of strips, then on the scalar ring once the tgt
    chunks have drained. The last two strips fan their writes across
    all three queues (sync is idle by then) to shorten the final drain.
    First src strip is loaded in quarters so the first matmuls start
    ~7 us after the fixed ~6.5 us engine preamble.
  - Fragmenting DMA below ~4 KB per-partition runs is catastrophic
    (573 B tgt row segments: +6 us; 288 B trimmed-write runs: +70 us);
    all transfers here keep >= 4.6 KB contiguous runs per partition.

Measured: ~42 MB HBM traffic/core at ~350 GB/s effective (HBM cap
~358 GB/s/core) -> memory-bound within ~10% of the achievable envelope
for this data layout. Run-to-run variance is +/- 4 us (8 cores share
HBM; contention phase varies).
"""

import sys

for _p in ("/opt/trn_rl_repo",):
    if _p not in sys.path:
        sys.path.insert(0, _p)

import numpy as np
import ml_dtypes

import concourse.mybir as mybir
import concourse.tile as tile
from concourse import bacc
from concourse.bass_utils import run_bass_kernel_spmd

B, C, S = 8, 128, 4
H, W = 160, 288
TY, TX = 16, 8                       # tile = 16x8 = 128 positions
GY = 4                               # y-rows per col-group (32 partitions)
NG = TY // GY                        # 4 col groups
WIN_X = TX + 2 * S                   # 16 window cols
BAND_Y = GY + 2 * S                  # 12 window rows per group band
BAND = BAND_Y * WIN_X                # 192 PSUM cols per tile
TGT_CHUNK = 8                        # tgt rows per chunk DMA
TQ = 4                               # position-tiles per PSUM allocation
PSB = 512                            # fp32 cols per 2 KiB PSUM bank
R_WIN = 120                          # rolling tgt window rows (ring in SBUF)
N_CORES = 8

BF16 = mybir.dt.bfloat16
NP_BF16 = ml_dtypes.bfloat16


def _displacements(s):
    d = [(0, 0)]
    for i in range(1, s + 1):
        d += [(-i, 0), (i, 0), (0, -i), (0, i)]
        for j in range(1, s + 1):
            d += [(-i, -j), (i, j), (-i, j), (i, -j)]
    return d


DISPLACEMENTS = _displacements(S)


def _build_bass(h=H, w=W, n_devices=N_CORES):
    nstrip = h // TY
    nxt = w // TX
    hp, wp = h + 2 * S, w + 2 * S
    n_chunks = (hp + TGT_CHUNK - 1) // TGT_CHUNK
    assert nxt % 2 == 0
    nsplit0 = 4 if nxt % 4 == 0 else 2   # src strip-0 / last-out split

    nc = bacc.Bacc(
        "TRN2",
        target_bir_lowering=False,
        debug=False,
        num_devices=n_devices,
    )
    # src pre-tiled on host: [C, nstrip, nxt*128] so each tile's lhsT is
    # one contiguous 128-element slice (pos = ylocal*8 + xlocal).
    src_t = nc.dram_tensor(
        "src", [C, nstrip, nxt * TY * TX], BF16, kind="ExternalInput"
    ).ap()
    tgt_t = nc.dram_tensor("tgtp", [C, h, wp], BF16, kind="ExternalInput").ap()
    out_t = nc.dram_tensor(
        "gram", [nstrip, C, nxt * BAND], BF16, kind="ExternalOutput"
    ).ap()

    with tile.TileContext(nc) as tc:
        with (
            tc.tile_pool(name="tgtres", bufs=1) as tgt_pool,
            tc.tile_pool(name="srcstrip", bufs=5) as src_pool,
            tc.tile_pool(name="outstrip", bufs=6) as out_pool,
            tc.tile_pool(name="psum", bufs=4, space="PSUM") as psum_pool,
        ):
            # Rolling 72-row tgt window: padded row r lives at SBUF row-slot
            # r % R_WIN. Strips read a 24-row band, so a 72-row ring gives
            # chunk loads a 2+-strip lead while using 28% of the SBUF of a
            # fully-resident tgt -- the freed space goes to out-buffers
            # (write-behind depth), which is what actually limits the late
            # phase: reads must front-load, so a ~6-8 MB write backlog
            # accumulates, and with shallow out-buffers the PSUM evacs
            # stall on buffer recycling. Tile's range-based hazard tracking
            # orders ring-slot reuse (WAR) automatically.
            tgt_tile = tgt_pool.tile([C, R_WIN * wp], BF16)
            tgt_view = tgt_tile.rearrange("p (y x) -> p y x", x=wp)
            # tgt is x-padded on the host; the S top pad rows (slots 0..S)
            # are memset here. The bottom pad (rows S+h..hp) shares slots
            # with earlier rows, so its memset is issued late, inside the
            # strip loop.
            nc.gpsimd.memset(tgt_view[:, 0:S], 0.0)
            def load_chunk(r0, r1, eng=None):
                ir0, ir1 = max(r0, S), min(r1, S + h)
                if ir0 < ir1:
                    sl = r0 % R_WIN
                    (eng or nc.scalar).dma_start(
                        tgt_view[:, sl + ir0 - r0 : sl + ir1 - r0],
                        tgt_t[:, ir0 - S : ir1 - S],
                    )

            # tgt arrives in 3 small 8-row chunks (strip 0's window, so the
            # first matmuls start fast) + 6 large 24-row chunks (always
            # slot-contiguous since 24 | 72). Few, large chunks matter:
            # HWDGE completion semaphores are a shared pool of ~8 recycled
            # round-robin, so issuing DMA k blocks the issuing ENGINE until
            # DMA k-8 completes. With 21 small chunks queued deep on the
            # scalar ring, those blocking waits delayed the ACT evacuations
            # behind them by 3-7us per strip, backing up PSUM and stalling
            # the tensor engine.
            chunks = [(0, 8), (8, 16), (16, 24)] + [
                (24 * (i + 1), 24 * (i + 2)) for i in range(6)
            ]
            assert chunks[-1][1] >= hp
            # Rows 0..120 have no ring-slot predecessor: issue up front.
            for r0, r1 in chunks[:7]:
                load_chunk(r0, r1)

            # Chunks reusing ring slots must be EMITTED after the last
            # strip (in program order) that reads the slots' previous
            # occupants -- the hazard tracker serializes in program order,
            # so emitting a slot-reusing chunk too early would order
            # earlier strips' reads AFTER the overwrite. Each entry keeps
            # a >=3-strip lead over the first strip needing the chunk. The
            # the last rides the sync ring (idle once src loads are issued)
            # so it doesn't queue behind the write backlog on scalar.
            chunk_at = {2: (7, None), 3: (8, nc.sync)}

            # Deferred out-piece issue: reads are the critical path (the
            # last strips cannot compute until their src lands), so writes
            # should soak up bandwidth LATE, after reads wind down, not
            # compete early. Pieces for strip s are issued at strip s+4's
            # top -- safely before the out pool recycles s's buffer at
            # strip s+6, and by then the evacs are long done so the issue
            # carries no blocking wait.
            pending_pieces: list = []

            def flush_pieces(now_s):
                while pending_pieces and (
                    now_s is None or pending_pieces[0][0] <= now_s
                ):
                    _, eng, ot, st, lo, hi = pending_pieces.pop(0)
                    eng.dma_start(
                        out_t[st, :, lo * BAND : hi * BAND],
                        ot[:, lo * BAND : hi * BAND],
                    )

            for s in range(nstrip):
                flush_pieces(s)
                src_tile = src_pool.tile([C, nxt * TY * TX], BF16)
                # First strip: quarters so the first matmuls start early.
                # Last two strips: their loads drain in the throttled late
                # window; quarters let each tile's matmuls start as soon
                # as its quarter lands instead of waiting for the full
                # 1.2 MB load to complete.
                nsplit = nsplit0 if (s == 0 or s >= nstrip - 2) else 1
                qt = nxt // nsplit * TY * TX
                for q in range(nsplit):
                    nc.sync.dma_start(
                        src_tile[:, q * qt : (q + 1) * qt],
                        src_t[:, s, q * qt : (q + 1) * qt],
                    )
                # chunk after the src load: a sync-ring chunk must not sit
                # ahead of an ungated src load in that FIFO
                if s in chunk_at:
                    ci, ceng = chunk_at[s]
                    load_chunk(*chunks[ci], eng=ceng)
                if s == 4:
                    # bottom pad rows S+h..hp sit at slots 44..48; their
                    # previous occupants (rows 44..48) are read through
                    # strip 2, and only strip 9 reads the pad.
                    nc.gpsimd.memset(
                        tgt_view[:, (S + h) % R_WIN : (S + h) % R_WIN + S],
                        0.0,
                    )
                src_view = src_tile.rearrange("p (t m) -> p t m", m=TY * TX)

                out_tile = out_pool.tile([C, nxt * BAND], BF16)

                # Out-DMA schedule: pieces issued mid-loop as soon as their
                # tiles are evacuated, so writes overlap the strip's own
                # compute. Early/mid strips: halves on one queue (gpsimd
                # SWDGE early, scalar once the tgt chunks have drained).
                # Last two strips: thirds fanned over gpsimd/sync/scalar
                # (sync's ring is idle once its last src load has issued).
                # Piece = 12 tiles keeps 4.6 KiB/partition contiguous runs.
                if s == nstrip - 1:
                    # final strip: quarters on the two HWDGE rings only --
                    # a SWDGE (gpsimd) write here would put the slow
                    # software-queue drain on the end-of-kernel barrier
                    pieces = [
                        (2, 0, 9, nc.scalar),
                        (4, 9, 18, nc.sync),
                        (6, 18, 27, nc.scalar),
                        (8, 27, 36, nc.sync),
                    ]
                elif s == nstrip - 2:
                    pieces = [
                        (2, 0, 12, nc.gpsimd),
                        (5, 12, 24, nc.sync),
                        (8, 24, 36, nc.scalar),
                    ]
                else:
                    # deferred: queued at evac, issued 4 strips later
                    pieces = [
                        (4, 0, 18, nc.gpsimd),
                        (8, 18, nxt, nc.scalar),
                    ]
                pi = 0
                # Deferring these issues was tried and REGRESSED (+6us):
                # removing early write traffic did not speed the reads
                # (they are paced by issue/semaphore mechanics, not pure
                # bandwidth arbitration), and the deferred bursts caused
                # new semaphore-recycle blocking on the issue engines.
                defer = False

                for tq in range(nxt // TQ):
                    # four position-tiles share one 2-bank PSUM allocation:
                    # two per bank (2*BAND=384 <= 512 fp32/bank), so each
                    # evacuation op moves 4 tiles and the ~350ns per-op
                    # engine overhead is paid 1/4 as often.
                    ps = psum_pool.tile([C, 2 * PSB], mybir.dt.float32)
                    ps_v = ps.rearrange("p (b c) -> p b c", c=PSB)
                    for ti in range(TQ):
                        t = TQ * tq + ti
                        off = PSB * (ti // 2) + BAND * (ti % 2)
                        for j in range(NG):
                            # group j's band rows live at ring slots
                            # sl..sl+12; when that range wraps the ring
                            # (strips 4 and 8, j pairs), split into two
                            # matmuls covering the psum col range piecewise
                            # (band row w -> psum cols 16w..16w+16).
                            sl = (TY * s + GY * j) % R_WIN
                            parts = (
                                [(sl, BAND_Y, 0)]
                                if sl + BAND_Y <= R_WIN
                                else [
                                    (sl, R_WIN - sl, 0),
                                    (0, BAND_Y - (R_WIN - sl), R_WIN - sl),
                                ]
                            )
                            for psl, nrow, w0 in parts:
                                nc.tensor.matmul(
                                    ps[
                                        32 * j : 32 * (j + 1),
                                        off + WIN_X * w0 : off
                                        + WIN_X * (w0 + nrow),
                                    ],
                                    lhsT=src_view[:, t, 32 * j : 32 * (j + 1)],
                                    rhs=tgt_view[
                                        :, psl : psl + nrow, TX * t : TX * t + WIN_X
                                    ],
                                    start=True,
                                    stop=True,
                                    tile_position=(0, 32 * j),
                                )
                    # Alternate evacuation engine so DVE and ACT each take
                    # half the quads and run concurrently. The source AP
                    # skips the 128-col pad at the top of each bank.
                    dst = out_tile[
                        :, TQ * tq * BAND : TQ * (tq + 1) * BAND
                    ].rearrange("p (b c) -> p b c", c=2 * BAND)
                    sap = ps_v[:, :, 0 : 2 * BAND]
                    if s == nstrip - 1 and tq >= nxt // TQ - 2:
                        # final quads: both engines, half each, to shorten
                        # the evac->last-write critical path
                        nc.vector.tensor_scalar_mul(dst[:, 0], sap[:, 0], 1.0 / C)
                        nc.scalar.mul(dst[:, 1], sap[:, 1], 1.0 / C)
                    elif tq % 2 == 0:
                        nc.vector.tensor_scalar_mul(dst, sap, 1.0 / C)
                    else:
                        nc.scalar.mul(dst, sap, 1.0 / C)
                    while pi < len(pieces) and pieces[pi][0] == tq:
                        _, lo, hi, out_eng = pieces[pi]
                        if defer:
                            pending_pieces.append(
                                (s + 4, out_eng, out_tile, s, lo, hi)
                            )
                        else:
                            out_eng.dma_start(
                                out_t[s, :, lo * BAND : hi * BAND],
                                out_tile[:, lo * BAND : hi * BAND],
                            )
                        pi += 1

            flush_pieces(None)

    nc.compile()
    return nc


_NC = None


def _get_nc():
    global _NC
    if _NC is None:
        _NC = _build_bass()
    return _NC


def _run_device(src_bf, tgtp_bf, **run_kwargs):
    nc = _get_nc()
    in_maps = [{"src": src_bf[b], "tgtp": tgtp_bf[b]} for b in range(B)]
    return run_bass_kernel_spmd(nc, in_maps, core_ids=list(range(N_CORES)), **run_kwargs)


def _pad_tgt(tgt, h=H, w=W):
    # x-padded only; the S top/bottom rows are memset on-device.
    b, c = tgt.shape[0], tgt.shape[1]
    tgtp = np.zeros((b, c, h, w + 2 * S), NP_BF16)
    tgtp[:, :, :, S : S + w] = tgt.astype(NP_BF16)
    return tgtp


def _pretile_src(src, h=H, w=W):
    """[B, C, h, w] -> [B, C, nstrip, nxt*TY*TX] bf16, pos = ylocal*TX+xlocal."""
    b, c = src.shape[0], src.shape[1]
    nstrip, nxt = h // TY, w // TX
    return np.ascontiguousarray(
        src.astype(NP_BF16)
        .reshape(b, c, nstrip, TY, nxt, TX)
        .transpose(0, 1, 2, 4, 3, 5)
        .reshape(b, c, nstrip, nxt * TY * TX)
    )


def _deshear(gram, h=H, w=W):
    """gram: [B, nstrip, 128, nxt*BAND] (any float dtype) -> [B, 81, h, w] fp32.

    gram[b, s, p, t*BAND + wr*WIN_X + wx] with p = ylocal*TX + xlocal holds
    (1/C) * sum_c src[c, TY*s+ylocal, TX*t+xlocal]
                * tgtp[c, TY*s + GY*(ylocal//GY) + wr, TX*t + wx]
    For displacement (dy,dx): wr = ylocal%GY + dy + S, wx = xlocal + dx + S.
    """
    b = gram.shape[0]
    nstrip, nxt = h // TY, w // TX
    g = np.asarray(gram, dtype=np.float32).reshape(
        b, nstrip, TY, TX, nxt, BAND_Y, WIN_X
    )
    out = np.empty((b, len(DISPLACEMENTS), h, w), np.float32)
    yy = np.arange(TY)[:, None]
    xx = np.arange(TX)[None, :]
    for k, (dy, dx) in enumerate(DISPLACEMENTS):
        # fancy dims (yy, xx) land first: v = [TY, TX, b, nstrip, nxt]
        v = g[:, :, yy, xx, :, (yy % GY) + dy + S, xx + dx + S]
        out[:, k] = v.transpose(2, 3, 0, 4, 1).reshape(b, h, w)
    return out


def kernel(src, tgt, _profile_out=None):
    src = np.asarray(src)
    tgt = np.asarray(tgt)
    assert src.shape == (B, C, H, W) and tgt.shape == (B, C, H, W)

    src_bf = _pretile_src(src)
    tgtp_bf = _pad_tgt(tgt)

    kw = {}
    if _profile_out is not None:
        kw["trace"] = True
        if _profile_out.get("tmpdir"):
            kw["tmpdir"] = _profile_out["tmpdir"]
    res = _run_device(src_bf, tgtp_bf, **kw)
    if _profile_out is not None:
        _profile_out.update(
            exec_time_ns=res.exec_time_ns,
            mean_exec_time_ns=res.mean_exec_time_ns,
        )

    gram = np.stack([res.results[b]["gram"] for b in range(B)])
    return _deshear(gram)

